# revision 1
# baseline (speedup 1.0000x reference)
"""Trainium2 Bass kernel for nn_CoarseGrainUpdate (gnn_message_passing).

Strategy (dictated by what this runtime supports — all Q7 custom DMA ops and
batched dynamic-AP gathers are broken/unavailable on this terminal):
  Launch A: scatter-mean numerator/denominator as a fixed-width padded
            segment reduction (Pool/DVE windowed reduce) on 8 cores,
            dst-range sharded. Division (max(cnt,1)) on device.
  Host:     index marshaling only — places pre-indexed operand rows into
            dense per-core grids (pure data movement, no arithmetic).
  Launch B: 8-way edge-sharded streaming compute: vec, norms, RBF (exp on
            ACT), spherical harmonics, and the [3,E,25] f32 output.
"""
import numpy as np
import concourse.bass as bass
import concourse.bacc as bacc
import concourse.tile as tile
import concourse.mybir as mybir
import concourse.bass_utils as bass_utils

N_CORES = 8
N_FRAME = 100000
N_TFN = 25000
E = 2000000
NUM_RBF = 16
EPS = 1e-8
SIGMA = 1.25           # (20-0)/16
MU = np.linspace(0.0, 20.0, NUM_RBF, dtype=np.float32)  # step 20/15
S3 = 1.7320508075688772
S5 = 2.23606797749979
S15 = 3.872983346207417

SEG_PAD = 25600                  # 25088 -> pad to 128*25*8
SEG_PER_CORE = SEG_PAD // N_CORES  # 3200
SEG_PER_PART = SEG_PER_CORE // 128  # 25
EDGES_PER_CORE = E // N_CORES    # 250000
CP = 1954                        # cols/partition: 128*1954 = 250112 >= 250000
EPC_PAD = 128 * CP

f32 = mybir.dt.float32

_cache = {}


def _build_launch_a(W):
    nc = bacc.Bacc("TRN2", target_bir_lowering=False, debug=False,
                   num_devices=N_CORES)
    FW = SEG_PER_PART * W
    grid_d = nc.dram_tensor("grid", [128, 4, FW], f32, kind="ExternalInput")
    out_d = nc.dram_tensor("tfn", [128, 3 * SEG_PER_PART], f32,
                           kind="ExternalOutput")
    P25 = SEG_PER_PART
    with tile.TileContext(nc) as tc:
        with tc.tile_pool(name="sbuf", bufs=1) as pool:
            g = pool.tile([128, 4, FW], f32)
            red = pool.tile([128, 4 * P25], f32)
            rec = pool.tile([128, P25], f32)
            o = pool.tile([128, 3 * P25], f32)
            nc.sync.dma_start(out=g[:], in_=grid_d.ap())
            # windowed segment reduction: [128, 4*P25, W] -> [128, 4*P25]
            nc.vector.tensor_reduce(
                red[:], g[:].rearrange("p c (s w) -> p (c s) w", w=W),
                axis=mybir.AxisListType.X, op=mybir.AluOpType.add)
            # denom = 1/max(cnt,1)
            nc.vector.tensor_scalar_max(rec[:], red[:, 3 * P25:4 * P25], 1.0)
            nc.vector.reciprocal(rec[:], rec[:])
            # tfn = sums * recip (broadcast over 3 channels)
            nc.vector.tensor_tensor(
                out=o[:], in0=red[:, 0:3 * P25],
                in1=rec[:].rearrange("p (o s) -> p o s", o=1).to_broadcast([128, 3, P25]),
                op=mybir.AluOpType.mult)
            nc.sync.dma_start(out=out_d.ap(), in_=o[:])
    nc.compile()
    return nc


def _build_launch_b():
    nc = bacc.Bacc("TRN2", target_bir_lowering=False, debug=False,
                   num_devices=N_CORES)
    ins = {}
    for t in range(3):
        ins[f"a{t}"] = nc.dram_tensor(f"a{t}", [128, CP, 3], f32,
                                      kind="ExternalInput")
        ins[f"b{t}"] = nc.dram_tensor(f"b{t}", [128, CP, 3], f32,
                                      kind="ExternalInput")
    mu_d = nc.dram_tensor("mu", [128, NUM_RBF], f32, kind="ExternalInput")
    out_d = nc.dram_tensor("out", [3, 128, CP * 25], f32,
                           kind="ExternalOutput")
    chunks = []
    i0 = 0
    while i0 < CP:
        c = min(256, CP - i0)
        chunks.append((i0, c))
        i0 += c
    with tile.TileContext(nc) as tc:
        with (tc.tile_pool(name="io", bufs=2) as iop,
              tc.tile_pool(name="wk", bufs=1) as wkp):
            mu_t = iop.tile([128, NUM_RBF], f32, tag="mu")
            nc.sync.dma_start(out=mu_t[:], in_=mu_d.ap())
            for t in range(3):
                for (i0, c) in chunks:
                    a = iop.tile([128, c, 3], f32, tag="a")
                    b = iop.tile([128, c, 3], f32, tag="b")
                    nc.sync.dma_start(out=a[:], in_=ins[f"a{t}"].ap()[:, i0:i0 + c, :])
                    nc.sync.dma_start(out=b[:], in_=ins[f"b{t}"].ap()[:, i0:i0 + c, :])
                    o = iop.tile([128, c, 25], f32, tag="o")
                    v = wkp.tile([128, c, 3], f32, tag="v")
                    se = wkp.tile([128, c, 3], f32, tag="se")
                    d2 = wkp.tile([128, c], f32, tag="d2")
                    d = wkp.tile([128, c], f32, tag="d")
                    inv = wkp.tile([128, c], f32, tag="inv")
                    r = wkp.tile([128, c, 3], f32, tag="r")
                    rs = wkp.tile([128, c, 3], f32, tag="rs")
                    u = wkp.tile([128, c, NUM_RBF], f32, tag="u")
                    tz = wkp.tile([128, c], f32, tag="tz")
                    ta = wkp.tile([128, c], f32, tag="ta")
                    tb = wkp.tile([128, c], f32, tag="tb")
                    sub = mybir.AluOpType.subtract
                    mul = mybir.AluOpType.mult
                    add = mybir.AluOpType.add
                    V = nc.vector
                    A = nc.scalar
                    V.tensor_tensor(out=v[:], in0=a[:], in1=b[:], op=sub)
                    V.tensor_scalar_add(se[:], v[:], EPS)
                    V.tensor_tensor(out=se[:], in0=se[:], in1=se[:], op=mul)
                    V.tensor_tensor(out=d2[:], in0=se[:, :, 0], in1=se[:, :, 1], op=add)
                    V.tensor_tensor(out=d2[:], in0=d2[:], in1=se[:, :, 2], op=add)
                    A.activation(d[:], d2[:], mybir.ActivationFunctionType.Sqrt)
                    V.reciprocal(inv[:], d[:])
                    V.tensor_tensor(
                        out=r[:], in0=v[:],
                        in1=inv[:].rearrange("p (c o) -> p c o", o=1).to_broadcast([128, c, 3]),
                        op=mul)
                    # RBF: exp(-((d-mu)/sigma)^2)
                    V.tensor_tensor(
                        out=u[:],
                        in0=d[:].rearrange("p (c o) -> p c o", o=1).to_broadcast([128, c, NUM_RBF]),
                        in1=mu_t[:].rearrange("p (o m) -> p o m", o=1).to_broadcast([128, c, NUM_RBF]),
                        op=sub)
                    A.activation(u[:], u[:], mybir.ActivationFunctionType.Square)
                    A.activation(o[:, :, 0:NUM_RBF], u[:],
                                 mybir.ActivationFunctionType.Exp,
                                 scale=-1.0 / (SIGMA * SIGMA))
                    # SH block
                    V.tensor_scalar(o[:, :, 16], d[:], 0.0, 1.0, op0=mul, op1=add)
                    A.activation(o[:, :, 17:20], r[:],
                                 mybir.ActivationFunctionType.Copy, scale=S3)
                    A.activation(rs[:], r[:],
                                 mybir.ActivationFunctionType.Copy, scale=S15)
                    V.tensor_tensor(out=o[:, :, 20], in0=r[:, :, 0], in1=rs[:, :, 1], op=mul)
                    V.tensor_tensor(out=o[:, :, 21], in0=r[:, :, 1], in1=rs[:, :, 2], op=mul)
                    V.tensor_tensor(out=o[:, :, 23], in0=r[:, :, 0], in1=rs[:, :, 2], op=mul)
                    V.tensor_tensor(out=tz[:], in0=r[:, :, 2], in1=rs[:, :, 2], op=mul)
                    V.tensor_scalar(o[:, :, 22], tz[:], 0.8660254037844386,
                                    -0.5 * S5, op0=mul, op1=add)
                    V.tensor_tensor(out=ta[:], in0=r[:, :, 0], in1=rs[:, :, 0], op=mul)
                    V.tensor_tensor(out=tb[:], in0=r[:, :, 1], in1=rs[:, :, 1], op=mul)
                    V.tensor_tensor(out=ta[:], in0=ta[:], in1=tb[:], op=sub)
                    V.tensor_scalar(o[:, :, 24], ta[:], 0.5, None, op0=mul)
                    nc.sync.dma_start(
                        out=out_d.ap()[t, :, i0 * 25:(i0 + c) * 25],
                        in_=o[:].rearrange("p c k -> p (c k)"))
    nc.compile()
    return nc


def _seg_grids(trans_g, f_src, t_dst, W):
    """Host marshaling: place trans[f_src] rows + mask into padded per-core
    channel-planar segment grids [N_CORES, 128, 4, SEG_PER_PART*W]."""
    n = f_src.shape[0]
    order = np.argsort(t_dst, kind="stable")
    sd = t_dst[order]
    sf = f_src[order]
    starts = np.searchsorted(sd, np.arange(N_TFN))
    rank = np.arange(n) - starts[sd]
    core = sd // SEG_PER_CORE
    local = sd % SEG_PER_CORE
    p = local // SEG_PER_PART
    j = local % SEG_PER_PART
    FW = SEG_PER_PART * W
    grids = np.zeros((N_CORES, 128, 4, FW), np.float32)
    vals = trans_g[sf]  # [n, 3]
    pos = j * W + rank
    grids[core, p, 0, pos] = vals[:, 0]
    grids[core, p, 1, pos] = vals[:, 1]
    grids[core, p, 2, pos] = vals[:, 2]
    grids[core, p, 3, pos] = 1.0
    return grids


def _edge_grid(rows):
    """[E_shard, 3] rows -> per-core [128, CP, 3] planar grids."""
    out = np.zeros((N_CORES, 128, CP, 3), np.float32)
    for k in range(N_CORES):
        shard = rows[k * EDGES_PER_CORE:(k + 1) * EDGES_PER_CORE]
        pad = np.zeros((EPC_PAD, 3), np.float32)
        pad[:EDGES_PER_CORE] = shard
        out[k] = pad.reshape(128, CP, 3)
    return out


def kernel(trans, frame2tfn_edge_index, tfn2tfn_edge_index,
           tfn2frame_edge_index, n_tfn):
    trans = np.asarray(trans, np.float32)
    f2t = np.asarray(frame2tfn_edge_index, np.int64)
    t2t = np.asarray(tfn2tfn_edge_index, np.int64)
    t2f = np.asarray(tfn2frame_edge_index, np.int64)

    f_src, t_dst = f2t[0], f2t[1]
    cnts = np.bincount(t_dst, minlength=N_TFN)
    W = int(cnts.max())

    # ---- Launch A: scatter-mean ----
    key = ("A", W)
    if key not in _cache:
        _cache[key] = _build_launch_a(W)
    ncA = _cache[key]
    grids = _seg_grids(trans, f_src, t_dst, W)
    in_maps = [{"grid": grids[k].reshape(128, 4, SEG_PER_PART * W)}
               for k in range(N_CORES)]
    resA = bass_utils.run_bass_kernel_spmd(ncA, in_maps,
                                           core_ids=list(range(N_CORES)))
    tfn_x = np.zeros((SEG_PAD, 3), np.float32)
    for k in range(N_CORES):
        o = resA.results[k]["tfn"].reshape(128, 3, SEG_PER_PART)
        segs = (np.arange(128)[:, None] * SEG_PER_PART
                + np.arange(SEG_PER_PART)[None, :] + k * SEG_PER_CORE)
        tfn_x[segs.ravel()] = o.transpose(0, 2, 1).reshape(-1, 3)
    tfn_x = tfn_x[:N_TFN]

    # ---- Host marshaling for Launch B ----
    a0 = _edge_grid(trans[f_src])
    b0 = _edge_grid(tfn_x[t_dst])
    a1 = _edge_grid(tfn_x[t2t[0]])
    b1 = _edge_grid(tfn_x[t2t[1]])
    a2 = _edge_grid(tfn_x[t2f[0]])
    b2 = _edge_grid(trans[t2f[1]])
    mu_grid = np.broadcast_to(MU[None, :], (128, NUM_RBF)).copy()

    # ---- Launch B: features ----
    if "B" not in _cache:
        _cache["B"] = _build_launch_b()
    ncB = _cache["B"]
    in_maps = [{"a0": a0[k], "b0": b0[k], "a1": a1[k], "b1": b1[k],
                "a2": a2[k], "b2": b2[k], "mu": mu_grid}
               for k in range(N_CORES)]
    resB = bass_utils.run_bass_kernel_spmd(ncB, in_maps,
                                           core_ids=list(range(N_CORES)))

    out = np.empty((3, E, NUM_RBF + 9), np.float32)
    for k in range(N_CORES):
        o = resB.results[k]["out"].reshape(3, EPC_PAD, 25)
        out[:, k * EDGES_PER_CORE:(k + 1) * EDGES_PER_CORE, :] = \
            o[:, :EDGES_PER_CORE, :]
    return out



# revision 4
# speedup vs baseline: 39281.6667x; 39281.6667x over previous
"""Trainium2 Bass kernel for nn_CoarseGrainUpdate (gnn_message_passing).

Strategy (dictated by what this runtime supports — Q7 custom DMA ops and
batched dynamic-AP gathers are broken/unavailable on this terminal):
  Launch A: scatter-mean numerator/denominator as a fixed-width padded
            segment reduction (DVE windowed reduce) on 8 cores,
            dst-range sharded. Division (max(cnt,1)) on device.
  Host:     index marshaling only — places pre-indexed operand rows into
            dense per-core grids (pure data movement, no arithmetic).
  Launch B: 8-way edge-sharded streaming compute: vec, norms, RBF (exp on
            ACT), spherical harmonics. Output stored as fp16 on device
            (error ~2^-11 relative, far inside the 2e-2 gate) and widened
            to f32 on the host — halves the dominant HBM write traffic.

Both launch builders accept repeat=N which wraps the body in a For_i
hardware loop; the graded path uses repeat=1 (no loop). test.py uses the
N>1 variants to measure steady-state device time independent of dispatch
overhead.
"""
import numpy as np
import concourse.bass as bass
import concourse.bacc as bacc
import concourse.tile as tile
import concourse.mybir as mybir
import concourse.bass_utils as bass_utils

N_CORES = 8
N_FRAME = 100000
N_TFN = 25000
E = 2000000
NUM_RBF = 16
EPS = 1e-8
SIGMA = 1.25           # (20-0)/16
MU = np.linspace(0.0, 20.0, NUM_RBF, dtype=np.float32)  # step 20/15
S3 = 1.7320508075688772
S5 = 2.23606797749979
S15 = 3.872983346207417

SEG_PAD = 25600                  # 25088 -> pad to 128*25*8
SEG_PER_CORE = SEG_PAD // N_CORES  # 3200
SEG_PER_PART = SEG_PER_CORE // 128  # 25
EDGES_PER_CORE = E // N_CORES    # 250000
CP = 1954                        # cols/partition: 128*1954 = 250112 >= 250000
EPC_PAD = 128 * CP
CHUNKS = (489, 489, 489, 487)    # sum = CP

f32 = mybir.dt.float32
f16 = mybir.dt.float16

_cache = {}


def _build_launch_a(W, repeat=1):
    nc = bacc.Bacc("TRN2", target_bir_lowering=False, debug=False,
                   num_devices=N_CORES)
    FW = SEG_PER_PART * W
    grid_d = nc.dram_tensor("grid", [128, 4, FW], f32, kind="ExternalInput")
    out_d = nc.dram_tensor("tfn", [128, 3 * SEG_PER_PART], f32,
                           kind="ExternalOutput")
    P25 = SEG_PER_PART
    with tile.TileContext(nc) as tc:
        with tc.tile_pool(name="sbuf", bufs=1) as pool:
            def body():
                g = pool.tile([128, 4, FW], f32, tag="g")
                red = pool.tile([128, 4 * P25], f32, tag="red")
                rec = pool.tile([128, P25], f32, tag="rec")
                o = pool.tile([128, 3 * P25], f32, tag="o")
                nc.sync.dma_start(out=g[:], in_=grid_d.ap())
                # windowed segment reduction: [128, 4*P25, W] -> [128, 4*P25]
                nc.vector.tensor_reduce(
                    red[:], g[:].rearrange("p c (s w) -> p (c s) w", w=W),
                    axis=mybir.AxisListType.X, op=mybir.AluOpType.add)
                # denom = 1/max(cnt,1)
                nc.vector.tensor_scalar_max(rec[:], red[:, 3 * P25:4 * P25], 1.0)
                nc.vector.reciprocal(rec[:], rec[:])
                # tfn = sums * recip (broadcast over 3 channels)
                nc.vector.tensor_tensor(
                    out=o[:], in0=red[:, 0:3 * P25],
                    in1=rec[:].rearrange("p (o s) -> p o s", o=1).to_broadcast([128, 3, P25]),
                    op=mybir.AluOpType.mult)
                nc.sync.dma_start(out=out_d.ap(), in_=o[:])
            if repeat == 1:
                body()
            else:
                with tc.For_i(0, repeat):
                    body()
    nc.compile()
    return nc


def _build_launch_b(repeat=1, use_gpsimd=True):
    nc = bacc.Bacc("TRN2", target_bir_lowering=False, debug=False,
                   num_devices=N_CORES)
    ins = {}
    for t in range(3):
        ins[f"ab{t}"] = nc.dram_tensor(f"ab{t}", [128, CP, 6], f32,
                                       kind="ExternalInput")
    mu_d = nc.dram_tensor("mu", [128, NUM_RBF], f32, kind="ExternalInput")
    out_d = nc.dram_tensor("out", [3, 128, CP * 25], f16,
                           kind="ExternalOutput")
    chunks = []
    i0 = 0
    for c in CHUNKS:
        chunks.append((i0, c))
        i0 += c

    sub = mybir.AluOpType.subtract
    mul = mybir.AluOpType.mult
    add = mybir.AluOpType.add
    ACTF = mybir.ActivationFunctionType

    with tile.TileContext(nc) as tc:
        with (tc.tile_pool(name="mup", bufs=1) as mup,
              tc.tile_pool(name="io", bufs=2) as iop,
              tc.tile_pool(name="wk", bufs=2) as wkp):
            mu_t = mup.tile([128, NUM_RBF], f32, tag="mu")
            nc.sync.dma_start(out=mu_t[:], in_=mu_d.ap())
            eps_t = mup.tile([128, 1], f32, tag="eps")
            nc.gpsimd.memset(eps_t[:], EPS)

            def body():
                V = nc.vector
                A = nc.scalar
                G = nc.gpsimd
                for t in range(3):
                    for (i0, c) in chunks:
                        ab = iop.tile([128, c, 6], f32, tag="ab")
                        o = iop.tile([128, c, 25], f16, tag="o")
                        nc.sync.dma_start(out=ab[:],
                                          in_=ins[f"ab{t}"].ap()[:, i0:i0 + c, :])
                        v = wkp.tile([128, c, 3], f32, tag="v")
                        se2 = wkp.tile([128, c, 3], f32, tag="se2")
                        d2 = wkp.tile([128, c], f32, tag="d2")
                        d = wkp.tile([128, c], f32, tag="d")
                        inv = wkp.tile([128, c], f32, tag="inv")
                        r = wkp.tile([128, c, 3], f32, tag="r")
                        u = wkp.tile([128, c, NUM_RBF], f32, tag="u")
                        zz = wkp.tile([128, c], f32, tag="zz")
                        p = wkp.tile([128, c], f32, tag="p")
                        m = wkp.tile([128, c], f32, tag="m")

                        # vec and eps-shifted squared norm
                        V.tensor_tensor(out=v[:], in0=ab[:, :, 0:3],
                                        in1=ab[:, :, 3:6], op=sub)
                        A.activation(se2[:], v[:], ACTF.Square, bias=eps_t[:])
                        V.tensor_reduce(d2[:], se2[:],
                                        axis=mybir.AxisListType.X,
                                        op=add)
                        A.activation(d[:], d2[:], ACTF.Sqrt)
                        V.reciprocal(inv[:], d[:])
                        V.tensor_tensor(
                            out=r[:], in0=v[:],
                            in1=inv[:].rearrange("p (c o) -> p c o", o=1).to_broadcast([128, c, 3]),
                            op=mul)
                        # RBF: exp(-((d-mu)/sigma)^2)
                        d_b = d[:].rearrange("p (c o) -> p c o", o=1).to_broadcast([128, c, NUM_RBF])
                        mu_b = mu_t[:].rearrange("p (o m) -> p o m", o=1).to_broadcast([128, c, NUM_RBF])
                        if use_gpsimd:
                            G.tensor_tensor(out=u[:], in0=d_b, in1=mu_b, op=sub)
                        else:
                            V.tensor_tensor(out=u[:], in0=d_b, in1=mu_b, op=sub)
                        V.tensor_tensor(out=u[:], in0=u[:], in1=u[:], op=mul)
                        A.activation(o[:, :, 0:NUM_RBF], u[:], ACTF.Exp,
                                     scale=-1.0 / (SIGMA * SIGMA))
                        # SH block
                        A.activation(o[:, :, 16], d[:], ACTF.Copy,
                                     scale=0.0, bias=1.0)
                        A.activation(o[:, :, 17:20], r[:], ACTF.Copy, scale=S3)
                        V.scalar_tensor_tensor(o[:, :, 20], r[:, :, 0], S15,
                                               r[:, :, 1], op0=mul, op1=mul)
                        V.scalar_tensor_tensor(o[:, :, 21], r[:, :, 1], S15,
                                               r[:, :, 2], op0=mul, op1=mul)
                        V.scalar_tensor_tensor(o[:, :, 23], r[:, :, 0], S15,
                                               r[:, :, 2], op0=mul, op1=mul)
                        V.tensor_tensor(out=zz[:], in0=r[:, :, 2],
                                        in1=r[:, :, 2], op=mul)
                        V.tensor_scalar(o[:, :, 22], zz[:], 1.5 * S5,
                                        -0.5 * S5, op0=mul, op1=add)
                        V.tensor_tensor(out=p[:], in0=r[:, :, 0],
                                        in1=r[:, :, 1], op=add)
                        V.tensor_tensor(out=m[:], in0=r[:, :, 0],
                                        in1=r[:, :, 1], op=sub)
                        V.scalar_tensor_tensor(o[:, :, 24], p[:], 0.5 * S15,
                                               m[:], op0=mul, op1=mul)
                        nc.sync.dma_start(
                            out=out_d.ap()[t, :, i0 * 25:(i0 + c) * 25],
                            in_=o[:].rearrange("p c k -> p (c k)"))
            if repeat == 1:
                body()
            else:
                with tc.For_i(0, repeat):
                    body()
    nc.compile()
    return nc


def _seg_grids(trans_g, f_src, t_dst, W):
    """Host marshaling: place trans[f_src] rows + mask into padded per-core
    channel-planar segment grids [N_CORES, 128, 4, SEG_PER_PART*W]."""
    n = f_src.shape[0]
    order = np.argsort(t_dst, kind="stable")
    sd = t_dst[order]
    sf = f_src[order]
    starts = np.searchsorted(sd, np.arange(N_TFN))
    rank = np.arange(n) - starts[sd]
    core = sd // SEG_PER_CORE
    local = sd % SEG_PER_CORE
    p = local // SEG_PER_PART
    j = local % SEG_PER_PART
    FW = SEG_PER_PART * W
    grids = np.zeros((N_CORES, 128, 4, FW), np.float32)
    vals = trans_g[sf]  # [n, 3]
    pos = j * W + rank
    grids[core, p, 0, pos] = vals[:, 0]
    grids[core, p, 1, pos] = vals[:, 1]
    grids[core, p, 2, pos] = vals[:, 2]
    grids[core, p, 3, pos] = 1.0
    return grids


def _edge_grid_ab(rows_a, rows_b):
    """[E, 3] a-rows and b-rows -> per-core [128, CP, 6] planar grids."""
    out = np.zeros((N_CORES, 128, CP, 6), np.float32)
    for k in range(N_CORES):
        pad = np.zeros((EPC_PAD, 6), np.float32)
        pad[:EDGES_PER_CORE, 0:3] = rows_a[k * EDGES_PER_CORE:(k + 1) * EDGES_PER_CORE]
        pad[:EDGES_PER_CORE, 3:6] = rows_b[k * EDGES_PER_CORE:(k + 1) * EDGES_PER_CORE]
        out[k] = pad.reshape(128, CP, 6)
    return out


def marshal_a(trans, f_src, t_dst):
    cnts = np.bincount(t_dst, minlength=N_TFN)
    W = int(cnts.max())
    grids = _seg_grids(trans, f_src, t_dst, W)
    in_maps = [{"grid": grids[k].reshape(128, 4, SEG_PER_PART * W)}
               for k in range(N_CORES)]
    return W, in_maps


def unshard_a(results):
    tfn_x = np.zeros((SEG_PAD, 3), np.float32)
    for k in range(N_CORES):
        o = results[k]["tfn"].reshape(128, 3, SEG_PER_PART)
        segs = (np.arange(128)[:, None] * SEG_PER_PART
                + np.arange(SEG_PER_PART)[None, :] + k * SEG_PER_CORE)
        tfn_x[segs.ravel()] = o.transpose(0, 2, 1).reshape(-1, 3)
    return tfn_x[:N_TFN]


def marshal_b(trans, tfn_x, f2t, t2t, t2f):
    f_src, t_dst = f2t[0], f2t[1]
    ab0 = _edge_grid_ab(trans[f_src], tfn_x[t_dst])
    ab1 = _edge_grid_ab(tfn_x[t2t[0]], tfn_x[t2t[1]])
    ab2 = _edge_grid_ab(tfn_x[t2f[0]], trans[t2f[1]])
    mu_grid = np.broadcast_to(MU[None, :], (128, NUM_RBF)).copy()
    return [{"ab0": ab0[k], "ab1": ab1[k], "ab2": ab2[k], "mu": mu_grid}
            for k in range(N_CORES)]


def unshard_b(results):
    out = np.empty((3, E, NUM_RBF + 9), np.float32)
    for k in range(N_CORES):
        o = results[k]["out"].astype(np.float32).reshape(3, EPC_PAD, 25)
        out[:, k * EDGES_PER_CORE:(k + 1) * EDGES_PER_CORE, :] = \
            o[:, :EDGES_PER_CORE, :]
    return out


def kernel(trans, frame2tfn_edge_index, tfn2tfn_edge_index,
           tfn2frame_edge_index, n_tfn):
    trans = np.asarray(trans, np.float32)
    f2t = np.asarray(frame2tfn_edge_index, np.int64)
    t2t = np.asarray(tfn2tfn_edge_index, np.int64)
    t2f = np.asarray(tfn2frame_edge_index, np.int64)

    # ---- Launch A: scatter-mean ----
    W, in_maps_a = marshal_a(trans, f2t[0], f2t[1])
    key = ("A", W, 1)
    if key not in _cache:
        _cache[key] = _build_launch_a(W)
    ncA = _cache[key]
    resA = bass_utils.run_bass_kernel_spmd(ncA, in_maps_a,
                                           core_ids=list(range(N_CORES)))
    tfn_x = unshard_a(resA.results)

    # ---- Launch B: features ----
    in_maps_b = marshal_b(trans, tfn_x, f2t, t2t, t2f)
    if ("B", 1) not in _cache:
        _cache[("B", 1)] = _build_launch_b()
    ncB = _cache[("B", 1)]
    resB = bass_utils.run_bass_kernel_spmd(ncB, in_maps_b,
                                           core_ids=list(range(N_CORES)))
    return unshard_b(resB.results)


# revision 39
# speedup vs baseline: 73738.6631x; 1.8772x over previous
"""Trainium2 Bass kernel for nn_CoarseGrainUpdate (gnn_message_passing).

Strategy (dictated by what this runtime supports — Q7 custom DMA ops and
batched dynamic-AP gathers are broken/unavailable on this terminal):
  Launch A: scatter-mean numerator/denominator as a fixed-width padded
            segment reduction (DVE windowed reduce) on 8 cores,
            dst-range sharded. Division (max(cnt,1)) on device.
  Host:     index marshaling only — places pre-indexed operand rows into
            dense per-core grids (pure data movement, no arithmetic).
  Launch B: 8-way edge-sharded streaming compute: vec, norms, RBF (exp on
            ACT), spherical harmonics. Output stored as fp16 on device
            (error ~2^-11 relative, far inside the 2e-2 gate) and widened
            to f32 on the host — halves the dominant HBM write traffic.

Both launch builders accept repeat=N which wraps the body in a For_i
hardware loop; the graded path uses repeat=1 (no loop). test.py uses the
N>1 variants to measure steady-state device time independent of dispatch
overhead.
"""
import numpy as np
import concourse.bass as bass
import concourse.bacc as bacc
import concourse.tile as tile
import concourse.mybir as mybir
import concourse.bass_utils as bass_utils

N_CORES = 8
N_FRAME = 100000
N_TFN = 25000
E = 2000000
NUM_RBF = 16
EPS = 1e-8
SIGMA = 1.25           # (20-0)/16
MU = np.linspace(0.0, 20.0, NUM_RBF, dtype=np.float32)  # step 20/15
S3 = 1.7320508075688772
S5 = 2.23606797749979
S15 = 3.872983346207417

SEG_PAD = 25600                  # 25088 -> pad to 128*25*8
SEG_PER_CORE = SEG_PAD // N_CORES  # 3200
SEG_PER_PART = SEG_PER_CORE // 128  # 25
EDGES_PER_CORE = E // N_CORES    # 250000
CP = 1954                        # cols/partition: 128*1954 = 250112 >= 250000
EPC_PAD = 128 * CP
CHUNKS = (489, 489, 489, 487)    # sum = CP

f32 = mybir.dt.float32
f16 = mybir.dt.float16

_cache = {}


def _build_launch_a(W, repeat=1):
    nc = bacc.Bacc("TRN2", target_bir_lowering=False, debug=False,
                   num_devices=N_CORES)
    FW = SEG_PER_PART * W
    grid_d = nc.dram_tensor("grid", [128, 4, FW], f32, kind="ExternalInput")
    out_d = nc.dram_tensor("tfn", [128, 3 * SEG_PER_PART], f32,
                           kind="ExternalOutput")
    P25 = SEG_PER_PART
    with tile.TileContext(nc) as tc:
        with tc.tile_pool(name="sbuf", bufs=1) as pool:
            def body():
                g = pool.tile([128, 4, FW], f32, tag="g")
                red = pool.tile([128, 4 * P25], f32, tag="red")
                rec = pool.tile([128, P25], f32, tag="rec")
                o = pool.tile([128, 3 * P25], f32, tag="o")
                nc.sync.dma_start(out=g[:], in_=grid_d.ap())
                # windowed segment reduction: [128, 4*P25, W] -> [128, 4*P25]
                nc.vector.tensor_reduce(
                    red[:], g[:].rearrange("p c (s w) -> p (c s) w", w=W),
                    axis=mybir.AxisListType.X, op=mybir.AluOpType.add)
                # denom = 1/max(cnt,1)
                nc.vector.tensor_scalar_max(rec[:], red[:, 3 * P25:4 * P25], 1.0)
                nc.vector.reciprocal(rec[:], rec[:])
                # tfn = sums * recip (broadcast over 3 channels)
                nc.vector.tensor_tensor(
                    out=o[:], in0=red[:, 0:3 * P25],
                    in1=rec[:].rearrange("p (o s) -> p o s", o=1).to_broadcast([128, 3, P25]),
                    op=mybir.AluOpType.mult)
                nc.sync.dma_start(out=out_d.ap(), in_=o[:])
            if repeat == 1:
                body()
            else:
                with tc.For_i(0, repeat):
                    body()
    nc.compile()
    return nc


def _build_launch_b(repeat=1, dma_only=False, unroll=False, rbf_j=8,
                    sq_act=False, use_dh=False, split_exp=True,
                    sh_eng="dve"):
    """Planar (feature-major) layout: per chunk the dram holds [128, 6, c]
    input blocks and [128, 25, c] fp16 output blocks so every engine op
    reads/writes contiguous runs. Two-stage software pipeline: front =
    DMA-in + norm chain (DVE+ACT), back = RBF tail (GPS+ACT) + SH (DVE) +
    DMA-out, with back(i-1) emitted before front(i) so each engine always
    has ready work queued in order."""
    nc = bacc.Bacc("TRN2", target_bir_lowering=False, debug=False,
                   num_devices=N_CORES)
    ins = {}
    for t in range(3):
        ins[f"ab{t}"] = nc.dram_tensor(f"ab{t}", [128, CP * 6], f32,
                                       kind="ExternalInput")
    mu_d = nc.dram_tensor("mu", [128, NUM_RBF], f32, kind="ExternalInput")
    out_d = nc.dram_tensor("out", [3, 128, CP * 25], f16,
                           kind="ExternalOutput")
    chunks = []
    i0 = 0
    for c in CHUNKS:
        chunks.append((i0, c))
        i0 += c

    sub = mybir.AluOpType.subtract
    mul = mybir.AluOpType.mult
    add = mybir.AluOpType.add
    ACTF = mybir.ActivationFunctionType
    V = nc.vector
    A = nc.scalar
    G = nc.gpsimd

    with tile.TileContext(nc) as tc:
        with (tc.tile_pool(name="mup", bufs=1) as mup,
              tc.tile_pool(name="io", bufs=2) as iop,
              tc.tile_pool(name="wk", bufs=2) as wkp):
            mu_t = mup.tile([128, NUM_RBF], f32, tag="mu")
            nc.sync.dma_start(out=mu_t[:], in_=mu_d.ap())
            # per-k RBF bias columns (-mu_k/sigma) and the 3*eps^2 sqrt bias
            rbf_b = mup.tile([128, NUM_RBF], f32, tag="rbf_b")
            for k in range(NUM_RBF):
                G.memset(rbf_b[:, k:k + 1], -float(MU[k]) / SIGMA)
            eps2_t = mup.tile([128, 1], f32, tag="eps2")
            G.memset(eps2_t[:], 3.0 * EPS * EPS)
            mu16_t = mup.tile([128, NUM_RBF], f16, tag="mu16")
            A.activation(mu16_t[:], mu_t[:], mybir.ActivationFunctionType.Copy)
            # f16 const tiles for GPS tensor_tensor finishing ops
            c_a = mup.tile([128, 1], f16, tag="c_a")      # 0.5*sqrt(5)
            G.memset(c_a[:], 0.5 * S5)
            c_half = mup.tile([128, 1], f16, tag="c_half")
            G.memset(c_half[:], 0.5)
            if dma_only:
                oc = mup.tile([128, 25 * CHUNKS[0]], f16, tag="oc")
                nc.gpsimd.memset(oc[:], 1.0)

            def dma_body():
                for t in range(3):
                    for (i0, c) in chunks:
                        ab = iop.tile([128, 6, c], f32, tag="ab")
                        nc.sync.dma_start(
                            out=ab[:],
                            in_=ins[f"ab{t}"].ap()[:, i0 * 6:(i0 + c) * 6]
                                .rearrange("p (s c) -> p s c", s=6))
                        nc.sync.dma_start(
                            out=out_d.ap()[t, :, i0 * 25:(i0 + c) * 25],
                            in_=oc[:, 0:25 * c])

            def front(t, i0, c):
                s = {}
                ab = iop.tile([128, 6, c], f32, tag="ab")
                nc.sync.dma_start(
                    out=ab[:],
                    in_=ins[f"ab{t}"].ap()[:, i0 * 6:(i0 + c) * 6]
                        .rearrange("p (s c) -> p s c", s=6))
                o = iop.tile([128, 25, c], f16, tag="o")
                v = wkp.tile([128, 3, c], f32, tag="v")
                sq = wkp.tile([128, 3, c], f32, tag="sq")
                d2 = wkp.tile([128, c], f32, tag="d2")
                d = wkp.tile([128, c], f32, tag="d")
                dh = wkp.tile([128, c], f16, tag="dh")
                inv = wkp.tile([128, c], f32, tag="inv")
                q = wkp.tile([128, NUM_RBF, c], f16, tag="q")
                V.tensor_tensor(out=v[:], in0=ab[:, 0:3, :], in1=ab[:, 3:6, :],
                                op=sub)
                if sq_act:
                    A.activation(sq[:], v[:], ACTF.Square)
                else:
                    V.tensor_tensor(out=sq[:], in0=v[:], in1=v[:], op=mul)
                V.tensor_tensor(out=d2[:], in0=sq[:, 0, :], in1=sq[:, 1, :],
                                op=add)
                V.tensor_tensor(out=d2[:], in0=d2[:], in1=sq[:, 2, :], op=add)
                # d = sqrt(sum(v^2) + 3*eps^2): identical to |v + eps| for
                # v == 0 exactly, and within ~1e-8 relative otherwise.
                A.activation(d[:], d2[:], ACTF.Sqrt, bias=eps2_t[:])
                V.reciprocal(inv[:], d[:])
                # l1 block written directly: rs = sqrt(3) * v / d  (fp16)
                inv_b = inv[:].rearrange("p (o c) -> p o c", o=1).to_broadcast([128, 3, c])
                V.scalar_tensor_tensor(o[:, 17:20, :], v[:], S3, inv_b,
                                       op0=mul, op1=mul)
                # RBF q_k = ((d - mu_k)/sigma)^2: k < rbf_j via per-k ACT
                # Square (bias port); the rest all-fp16 on DVE (2x mode):
                # dh = d/sigma (fp16), u = dh - mu16, q = u*u in place.
                for k in range(rbf_j):
                    A.activation(q[:, k, :], d[:], ACTF.Square,
                                 scale=1.0 / SIGMA, bias=rbf_b[:, k:k + 1])
                if rbf_j < NUM_RBF:
                    nk = NUM_RBF - rbf_j
                    if use_dh:
                        V.tensor_scalar_mul(dh[:], d[:], 1.0 / SIGMA)
                        dh_b = dh[:].rearrange("p (o c) -> p o c", o=1).to_broadcast([128, nk, c])
                        mu_b = mu16_t[:, rbf_j:].rearrange("p (m o) -> p m o", o=1).to_broadcast([128, nk, c])
                        V.tensor_tensor(out=q[:, rbf_j:, :], in0=dh_b,
                                        in1=mu_b, op=sub)
                    else:
                        d_b = d[:].rearrange("p (o c) -> p o c", o=1).to_broadcast([128, nk, c])
                        mu_b = mu_t[:, rbf_j:].rearrange("p (m o) -> p m o", o=1).to_broadcast([128, nk, c])
                        V.scalar_tensor_tensor(q[:, rbf_j:, :], d_b,
                                               1.0 / SIGMA, mu_b,
                                               op0=mul, op1=sub)
                    V.tensor_tensor(out=q[:, rbf_j:, :], in0=q[:, rbf_j:, :],
                                    in1=q[:, rbf_j:, :], op=mul)
                s["o"], s["q"] = o, q
                return s

            def back(t, i0, c, s):
                o, q = s["o"], s["q"]
                if split_exp and 0 < rbf_j < NUM_RBF:
                    # split: ACT-produced q rows don't wait on the DVE rows
                    A.activation(o[:, 0:rbf_j, :], q[:, 0:rbf_j, :],
                                 ACTF.Exp, scale=-1.0)
                    A.activation(o[:, rbf_j:NUM_RBF, :], q[:, rbf_j:, :],
                                 ACTF.Exp, scale=-1.0)
                else:
                    A.activation(o[:, 0:NUM_RBF, :], q[:], ACTF.Exp,
                                 scale=-1.0)
                G.memset(o[:, 16, :], 1.0)
                rs = o[:, 17:20, :]
                if sh_eng == "gps":
                    # SH l2 via GPS tensor_tensor only (TensorScalar/STT are
                    # illegal on Pool): pre-scaled copies make every l2
                    # output a plain product. rss = sqrt(S15/3)*rs.
                    rss = wkp.tile([128, 3, c], f16, tag="rss")
                    rq2 = wkp.tile([128, c], f16, tag="rq2")
                    V.tensor_scalar_mul(rss[:], rs[:],
                                        float(np.sqrt(S15 / 3.0)))
                    V.tensor_scalar_mul(rq2[:], rs[:, 2, :],
                                        float(np.sqrt(0.5 * S5)))
                    zz = wkp.tile([128, c], f16, tag="zz")
                    p = wkp.tile([128, c], f16, tag="p")
                    m = wkp.tile([128, c], f16, tag="m")
                    t24 = wkp.tile([128, c], f16, tag="t24")
                    G.tensor_tensor(out=o[:, 20, :], in0=rss[:, 0, :],
                                    in1=rss[:, 1, :], op=mul)
                    G.tensor_tensor(out=o[:, 21, :], in0=rss[:, 1, :],
                                    in1=rss[:, 2, :], op=mul)
                    G.tensor_tensor(out=o[:, 23, :], in0=rss[:, 0, :],
                                    in1=rss[:, 2, :], op=mul)
                    G.tensor_tensor(out=zz[:], in0=rq2[:], in1=rq2[:], op=mul)
                    G.tensor_tensor(out=o[:, 22, :], in0=zz[:],
                                    in1=c_a[:].to_broadcast([128, c]), op=sub)
                    G.tensor_tensor(out=p[:], in0=rss[:, 0, :],
                                    in1=rss[:, 1, :], op=add)
                    G.tensor_tensor(out=m[:], in0=rss[:, 0, :],
                                    in1=rss[:, 1, :], op=sub)
                    G.tensor_tensor(out=t24[:], in0=p[:], in1=m[:], op=mul)
                    G.tensor_tensor(out=o[:, 24, :], in0=t24[:],
                                    in1=c_half[:].to_broadcast([128, c]),
                                    op=mul)
                else:
                    V.scalar_tensor_tensor(o[:, 20, :], rs[:, 0, :],
                                           S15 / 3.0, rs[:, 1, :],
                                           op0=mul, op1=mul)
                    V.scalar_tensor_tensor(o[:, 21, :], rs[:, 1, :],
                                           S15 / 3.0, rs[:, 2, :],
                                           op0=mul, op1=mul)
                    V.scalar_tensor_tensor(o[:, 23, :], rs[:, 0, :],
                                           S15 / 3.0, rs[:, 2, :],
                                           op0=mul, op1=mul)
                    zz = wkp.tile([128, c], f16, tag="zz")
                    p = wkp.tile([128, c], f16, tag="p")
                    m = wkp.tile([128, c], f16, tag="m")
                    V.tensor_tensor(out=zz[:], in0=rs[:, 2, :],
                                    in1=rs[:, 2, :], op=mul)
                    V.tensor_scalar(o[:, 22, :], zz[:], 0.5 * S5, -0.5 * S5,
                                    op0=mul, op1=add)
                    V.tensor_tensor(out=p[:], in0=rs[:, 0, :],
                                    in1=rs[:, 1, :], op=add)
                    V.tensor_tensor(out=m[:], in0=rs[:, 0, :],
                                    in1=rs[:, 1, :], op=sub)
                    V.scalar_tensor_tensor(o[:, 24, :], p[:], 0.5 * S15 / 3.0,
                                           m[:], op0=mul, op1=mul)
                nc.sync.dma_start(
                    out=out_d.ap()[t, :, i0 * 25:(i0 + c) * 25],
                    in_=o[:].rearrange("p k c -> p (k c)"))

            def body():
                prev = None
                for t in range(3):
                    for (i0, c) in chunks:
                        if prev is not None:
                            back(*prev)
                        prev = (t, i0, c, front(t, i0, c))
                back(*prev)

            use_body = dma_body if dma_only else body
            if repeat == 1:
                use_body()
            elif unroll:
                for _ in range(repeat):
                    use_body()
            else:
                with tc.For_i(0, repeat):
                    use_body()
    nc.compile()
    return nc


def _seg_grids(trans_g, f_src, t_dst, W):
    """Host marshaling: place trans[f_src] rows + mask into padded per-core
    channel-planar segment grids [N_CORES, 128, 4, SEG_PER_PART*W]."""
    n = f_src.shape[0]
    order = np.argsort(t_dst, kind="stable")
    sd = t_dst[order]
    sf = f_src[order]
    starts = np.searchsorted(sd, np.arange(N_TFN))
    rank = np.arange(n) - starts[sd]
    core = sd // SEG_PER_CORE
    local = sd % SEG_PER_CORE
    p = local // SEG_PER_PART
    j = local % SEG_PER_PART
    FW = SEG_PER_PART * W
    grids = np.zeros((N_CORES, 128, 4, FW), np.float32)
    vals = trans_g[sf]  # [n, 3]
    pos = j * W + rank
    grids[core, p, 0, pos] = vals[:, 0]
    grids[core, p, 1, pos] = vals[:, 1]
    grids[core, p, 2, pos] = vals[:, 2]
    grids[core, p, 3, pos] = 1.0
    return grids


def _edge_grid_ab(rows_a, rows_b):
    """[E, 3] a-rows and b-rows -> per-core [128, CP*6] grids laid out as
    per-chunk planar [128, 6, c] blocks (matching the device AP)."""
    out = np.empty((N_CORES, 128, CP * 6), np.float32)
    for k in range(N_CORES):
        pad = np.zeros((EPC_PAD, 6), np.float32)
        pad[:EDGES_PER_CORE, 0:3] = rows_a[k * EDGES_PER_CORE:(k + 1) * EDGES_PER_CORE]
        pad[:EDGES_PER_CORE, 3:6] = rows_b[k * EDGES_PER_CORE:(k + 1) * EDGES_PER_CORE]
        g = pad.reshape(128, CP, 6)
        i0 = 0
        for c in CHUNKS:
            blk = g[:, i0:i0 + c, :].transpose(0, 2, 1)  # [128, 6, c]
            out[k][:, i0 * 6:(i0 + c) * 6] = blk.reshape(128, 6 * c)
            i0 += c
    return out


def marshal_a(trans, f_src, t_dst):
    cnts = np.bincount(t_dst, minlength=N_TFN)
    W = int(cnts.max())
    grids = _seg_grids(trans, f_src, t_dst, W)
    in_maps = [{"grid": grids[k].reshape(128, 4, SEG_PER_PART * W)}
               for k in range(N_CORES)]
    return W, in_maps


def unshard_a(results):
    tfn_x = np.zeros((SEG_PAD, 3), np.float32)
    for k in range(N_CORES):
        o = results[k]["tfn"].reshape(128, 3, SEG_PER_PART)
        segs = (np.arange(128)[:, None] * SEG_PER_PART
                + np.arange(SEG_PER_PART)[None, :] + k * SEG_PER_CORE)
        tfn_x[segs.ravel()] = o.transpose(0, 2, 1).reshape(-1, 3)
    return tfn_x[:N_TFN]


def marshal_b(trans, tfn_x, f2t, t2t, t2f):
    f_src, t_dst = f2t[0], f2t[1]
    ab0 = _edge_grid_ab(trans[f_src], tfn_x[t_dst])
    ab1 = _edge_grid_ab(tfn_x[t2t[0]], tfn_x[t2t[1]])
    ab2 = _edge_grid_ab(tfn_x[t2f[0]], trans[t2f[1]])
    mu_grid = np.broadcast_to((MU / SIGMA)[None, :], (128, NUM_RBF)).copy()
    return [{"ab0": ab0[k], "ab1": ab1[k], "ab2": ab2[k], "mu": mu_grid}
            for k in range(N_CORES)]


def unshard_b(results):
    out = np.empty((3, E, NUM_RBF + 9), np.float32)
    for k in range(N_CORES):
        o = results[k]["out"].astype(np.float32)  # [3, 128, CP*25]
        full = np.empty((3, 128, CP, 25), np.float32)
        i0 = 0
        for c in CHUNKS:
            blk = o[:, :, i0 * 25:(i0 + c) * 25].reshape(3, 128, 25, c)
            full[:, :, i0:i0 + c, :] = blk.transpose(0, 1, 3, 2)
            i0 += c
        out[:, k * EDGES_PER_CORE:(k + 1) * EDGES_PER_CORE, :] = \
            full.reshape(3, EPC_PAD, 25)[:, :EDGES_PER_CORE, :]
    return out


def kernel(trans, frame2tfn_edge_index, tfn2tfn_edge_index,
           tfn2frame_edge_index, n_tfn):
    trans = np.asarray(trans, np.float32)
    f2t = np.asarray(frame2tfn_edge_index, np.int64)
    t2t = np.asarray(tfn2tfn_edge_index, np.int64)
    t2f = np.asarray(tfn2frame_edge_index, np.int64)

    # ---- Launch A: scatter-mean ----
    W, in_maps_a = marshal_a(trans, f2t[0], f2t[1])
    key = ("A", W, 1)
    if key not in _cache:
        _cache[key] = _build_launch_a(W)
    ncA = _cache[key]
    resA = bass_utils.run_bass_kernel_spmd(ncA, in_maps_a,
                                           core_ids=list(range(N_CORES)))
    tfn_x = unshard_a(resA.results)

    # ---- Launch B: features ----
    in_maps_b = marshal_b(trans, tfn_x, f2t, t2t, t2f)
    if ("B", 1) not in _cache:
        _cache[("B", 1)] = _build_launch_b()
    ncB = _cache[("B", 1)]
    resB = bass_utils.run_bass_kernel_spmd(ncB, in_maps_b,
                                           core_ids=list(range(N_CORES)))
    return unshard_b(resB.results)


# revision 42
# speedup vs baseline: 84138.1265x; 1.1410x over previous
"""Trainium2 Bass kernel for nn_CoarseGrainUpdate (gnn_message_passing).

Strategy (dictated by what this runtime supports — Q7 custom DMA ops and
batched dynamic-AP gathers are broken/unavailable on this terminal):
  Launch A: scatter-mean numerator/denominator as a fixed-width padded
            segment reduction (DVE windowed reduce) on 8 cores,
            dst-range sharded. Division (max(cnt,1)) on device.
  Host:     index marshaling only — places pre-indexed operand rows into
            dense per-core grids (pure data movement, no arithmetic).
  Launch B: 8-way edge-sharded streaming compute: vec, norms, RBF (exp on
            ACT), spherical harmonics. Output stored as fp16 on device
            (error ~2^-11 relative, far inside the 2e-2 gate) and widened
            to f32 on the host — halves the dominant HBM write traffic.

Both launch builders accept repeat=N which wraps the body in a For_i
hardware loop; the graded path uses repeat=1 (no loop). test.py uses the
N>1 variants to measure steady-state device time independent of dispatch
overhead.
"""
import numpy as np
import concourse.bass as bass
import concourse.bacc as bacc
import concourse.tile as tile
import concourse.mybir as mybir
import concourse.bass_utils as bass_utils

N_CORES = 8
N_FRAME = 100000
N_TFN = 25000
E = 2000000
NUM_RBF = 16
EPS = 1e-8
SIGMA = 1.25           # (20-0)/16
MU = np.linspace(0.0, 20.0, NUM_RBF, dtype=np.float32)  # step 20/15
S3 = 1.7320508075688772
S5 = 2.23606797749979
S15 = 3.872983346207417

SEG_PAD = 25600                  # 25088 -> pad to 128*25*8
SEG_PER_CORE = SEG_PAD // N_CORES  # 3200
SEG_PER_PART = SEG_PER_CORE // 128  # 25
EDGES_PER_CORE = E // N_CORES    # 250000
CP = 1954                        # cols/partition: 128*1954 = 250112 >= 250000
EPC_PAD = 128 * CP
CHUNKS = (651, 651, 652)         # sum = CP

f32 = mybir.dt.float32
f16 = mybir.dt.float16

_cache = {}


def _build_launch_a(W, repeat=1):
    nc = bacc.Bacc("TRN2", target_bir_lowering=False, debug=False,
                   num_devices=N_CORES)
    FW = SEG_PER_PART * W
    grid_d = nc.dram_tensor("grid", [128, 4, FW], f32, kind="ExternalInput")
    out_d = nc.dram_tensor("tfn", [128, 3 * SEG_PER_PART], f32,
                           kind="ExternalOutput")
    P25 = SEG_PER_PART
    SBLK = (7, 6, 6, 6)  # segment blocks: sum = 25, pipelined DMA/reduce
    with tile.TileContext(nc) as tc:
        with (tc.tile_pool(name="io", bufs=2) as iop,
              tc.tile_pool(name="sbuf", bufs=1) as pool):
            def body():
                red = pool.tile([128, 4, P25], f32, tag="red")
                rec = pool.tile([128, P25], f32, tag="rec")
                o = pool.tile([128, 3 * P25], f32, tag="o")
                s0 = 0
                for sc in SBLK:
                    g = iop.tile([128, 4, sc * W], f32, tag="g")
                    nc.sync.dma_start(
                        out=g[:], in_=grid_d.ap()[:, :, s0 * W:(s0 + sc) * W])
                    nc.vector.tensor_reduce(
                        red[:, :, s0:s0 + sc],
                        g[:].rearrange("p c (s w) -> p (c s) w", w=W),
                        axis=mybir.AxisListType.X, op=mybir.AluOpType.add)
                    s0 += sc
                redf = red[:].rearrange("p c s -> p (c s)")
                # denom = 1/max(cnt,1)
                nc.vector.tensor_scalar_max(rec[:], redf[:, 3 * P25:4 * P25],
                                            1.0)
                nc.vector.reciprocal(rec[:], rec[:])
                # tfn = sums * recip (broadcast over 3 channels)
                nc.vector.tensor_tensor(
                    out=o[:], in0=redf[:, 0:3 * P25],
                    in1=rec[:].rearrange("p (o s) -> p o s", o=1).to_broadcast([128, 3, P25]),
                    op=mybir.AluOpType.mult)
                nc.sync.dma_start(out=out_d.ap(), in_=o[:])
            if repeat == 1:
                body()
            else:
                with tc.For_i(0, repeat):
                    body()
    nc.compile()
    return nc


def _build_launch_b(repeat=1, dma_only=False, unroll=False, rbf_j=12,
                    sq_act=False, use_dh=False, split_exp=True,
                    sh_eng="dve"):
    """Planar (feature-major) layout: per chunk the dram holds [128, 6, c]
    input blocks and [128, 25, c] fp16 output blocks so every engine op
    reads/writes contiguous runs. Two-stage software pipeline: front =
    DMA-in + norm chain (DVE+ACT), back = RBF tail (GPS+ACT) + SH (DVE) +
    DMA-out, with back(i-1) emitted before front(i) so each engine always
    has ready work queued in order."""
    nc = bacc.Bacc("TRN2", target_bir_lowering=False, debug=False,
                   num_devices=N_CORES)
    ins = {}
    for t in range(3):
        ins[f"ab{t}"] = nc.dram_tensor(f"ab{t}", [128, CP * 6], f32,
                                       kind="ExternalInput")
    mu_d = nc.dram_tensor("mu", [128, NUM_RBF], f32, kind="ExternalInput")
    out_d = nc.dram_tensor("out", [3, 128, CP * 25], f16,
                           kind="ExternalOutput")
    chunks = []
    i0 = 0
    for c in CHUNKS:
        chunks.append((i0, c))
        i0 += c

    sub = mybir.AluOpType.subtract
    mul = mybir.AluOpType.mult
    add = mybir.AluOpType.add
    ACTF = mybir.ActivationFunctionType
    V = nc.vector
    A = nc.scalar
    G = nc.gpsimd

    with tile.TileContext(nc) as tc:
        with (tc.tile_pool(name="mup", bufs=1) as mup,
              tc.tile_pool(name="io", bufs=2) as iop,
              tc.tile_pool(name="wk", bufs=2) as wkp):
            mu_t = mup.tile([128, NUM_RBF], f32, tag="mu")
            nc.sync.dma_start(out=mu_t[:], in_=mu_d.ap())
            # per-k RBF bias columns (-mu_k/sigma) and the 3*eps^2 sqrt bias
            rbf_b = mup.tile([128, NUM_RBF], f32, tag="rbf_b")
            for k in range(NUM_RBF):
                G.memset(rbf_b[:, k:k + 1], -float(MU[k]) / SIGMA)
            eps2_t = mup.tile([128, 1], f32, tag="eps2")
            G.memset(eps2_t[:], 3.0 * EPS * EPS)
            mu16_t = mup.tile([128, NUM_RBF], f16, tag="mu16")
            A.activation(mu16_t[:], mu_t[:], mybir.ActivationFunctionType.Copy)
            # f16 const tiles for GPS tensor_tensor finishing ops
            c_a = mup.tile([128, 1], f16, tag="c_a")      # 0.5*sqrt(5)
            G.memset(c_a[:], 0.5 * S5)
            c_half = mup.tile([128, 1], f16, tag="c_half")
            G.memset(c_half[:], 0.5)
            if dma_only:
                oc = mup.tile([128, 25 * CHUNKS[0]], f16, tag="oc")
                nc.gpsimd.memset(oc[:], 1.0)

            def dma_body():
                for t in range(3):
                    for (i0, c) in chunks:
                        ab = iop.tile([128, 6, c], f32, tag="ab")
                        nc.sync.dma_start(
                            out=ab[:],
                            in_=ins[f"ab{t}"].ap()[:, i0 * 6:(i0 + c) * 6]
                                .rearrange("p (s c) -> p s c", s=6))
                        nc.sync.dma_start(
                            out=out_d.ap()[t, :, i0 * 25:(i0 + c) * 25],
                            in_=oc[:, 0:25 * c])

            def front(t, i0, c):
                s = {}
                ab = iop.tile([128, 6, c], f32, tag="ab")
                nc.sync.dma_start(
                    out=ab[:],
                    in_=ins[f"ab{t}"].ap()[:, i0 * 6:(i0 + c) * 6]
                        .rearrange("p (s c) -> p s c", s=6))
                o = iop.tile([128, 25, c], f16, tag="o")
                v = wkp.tile([128, 3, c], f32, tag="v")
                sq = wkp.tile([128, 3, c], f32, tag="sq")
                d2 = wkp.tile([128, c], f32, tag="d2")
                d = wkp.tile([128, c], f32, tag="d")
                dh = wkp.tile([128, c], f16, tag="dh")
                inv = wkp.tile([128, c], f32, tag="inv")
                q = wkp.tile([128, NUM_RBF, c], f16, tag="q")
                V.tensor_tensor(out=v[:], in0=ab[:, 0:3, :], in1=ab[:, 3:6, :],
                                op=sub)
                if sq_act:
                    A.activation(sq[:], v[:], ACTF.Square)
                else:
                    V.tensor_tensor(out=sq[:], in0=v[:], in1=v[:], op=mul)
                V.tensor_tensor(out=d2[:], in0=sq[:, 0, :], in1=sq[:, 1, :],
                                op=add)
                V.tensor_tensor(out=d2[:], in0=d2[:], in1=sq[:, 2, :], op=add)
                # d = sqrt(sum(v^2) + 3*eps^2): identical to |v + eps| for
                # v == 0 exactly, and within ~1e-8 relative otherwise.
                A.activation(d[:], d2[:], ACTF.Sqrt, bias=eps2_t[:])
                V.reciprocal(inv[:], d[:])
                # l1 block written directly: rs = sqrt(3) * v / d  (fp16)
                inv_b = inv[:].rearrange("p (o c) -> p o c", o=1).to_broadcast([128, 3, c])
                V.scalar_tensor_tensor(o[:, 17:20, :], v[:], S3, inv_b,
                                       op0=mul, op1=mul)
                # RBF q_k = ((d - mu_k)/sigma)^2: k < rbf_j via per-k ACT
                # Square (bias port); the rest all-fp16 on DVE (2x mode):
                # dh = d/sigma (fp16), u = dh - mu16, q = u*u in place.
                for k in range(rbf_j):
                    A.activation(q[:, k, :], d[:], ACTF.Square,
                                 scale=1.0 / SIGMA, bias=rbf_b[:, k:k + 1])
                if rbf_j < NUM_RBF:
                    nk = NUM_RBF - rbf_j
                    if use_dh:
                        V.tensor_scalar_mul(dh[:], d[:], 1.0 / SIGMA)
                        dh_b = dh[:].rearrange("p (o c) -> p o c", o=1).to_broadcast([128, nk, c])
                        mu_b = mu16_t[:, rbf_j:].rearrange("p (m o) -> p m o", o=1).to_broadcast([128, nk, c])
                        V.tensor_tensor(out=q[:, rbf_j:, :], in0=dh_b,
                                        in1=mu_b, op=sub)
                    else:
                        d_b = d[:].rearrange("p (o c) -> p o c", o=1).to_broadcast([128, nk, c])
                        mu_b = mu_t[:, rbf_j:].rearrange("p (m o) -> p m o", o=1).to_broadcast([128, nk, c])
                        V.scalar_tensor_tensor(q[:, rbf_j:, :], d_b,
                                               1.0 / SIGMA, mu_b,
                                               op0=mul, op1=sub)
                    V.tensor_tensor(out=q[:, rbf_j:, :], in0=q[:, rbf_j:, :],
                                    in1=q[:, rbf_j:, :], op=mul)
                s["o"], s["q"] = o, q
                return s

            def back(t, i0, c, s):
                o, q = s["o"], s["q"]
                if split_exp and 0 < rbf_j < NUM_RBF:
                    # split: ACT-produced q rows don't wait on the DVE rows
                    A.activation(o[:, 0:rbf_j, :], q[:, 0:rbf_j, :],
                                 ACTF.Exp, scale=-1.0)
                    A.activation(o[:, rbf_j:NUM_RBF, :], q[:, rbf_j:, :],
                                 ACTF.Exp, scale=-1.0)
                else:
                    A.activation(o[:, 0:NUM_RBF, :], q[:], ACTF.Exp,
                                 scale=-1.0)
                G.memset(o[:, 16, :], 1.0)
                rs = o[:, 17:20, :]
                if sh_eng == "gps":
                    # SH l2 via GPS tensor_tensor only (TensorScalar/STT are
                    # illegal on Pool): pre-scaled copies make every l2
                    # output a plain product. rss = sqrt(S15/3)*rs.
                    rss = wkp.tile([128, 3, c], f16, tag="rss")
                    rq2 = wkp.tile([128, c], f16, tag="rq2")
                    V.tensor_scalar_mul(rss[:], rs[:],
                                        float(np.sqrt(S15 / 3.0)))
                    V.tensor_scalar_mul(rq2[:], rs[:, 2, :],
                                        float(np.sqrt(0.5 * S5)))
                    zz = wkp.tile([128, c], f16, tag="zz")
                    p = wkp.tile([128, c], f16, tag="p")
                    m = wkp.tile([128, c], f16, tag="m")
                    t24 = wkp.tile([128, c], f16, tag="t24")
                    G.tensor_tensor(out=o[:, 20, :], in0=rss[:, 0, :],
                                    in1=rss[:, 1, :], op=mul)
                    G.tensor_tensor(out=o[:, 21, :], in0=rss[:, 1, :],
                                    in1=rss[:, 2, :], op=mul)
                    G.tensor_tensor(out=o[:, 23, :], in0=rss[:, 0, :],
                                    in1=rss[:, 2, :], op=mul)
                    G.tensor_tensor(out=zz[:], in0=rq2[:], in1=rq2[:], op=mul)
                    G.tensor_tensor(out=o[:, 22, :], in0=zz[:],
                                    in1=c_a[:].to_broadcast([128, c]), op=sub)
                    G.tensor_tensor(out=p[:], in0=rss[:, 0, :],
                                    in1=rss[:, 1, :], op=add)
                    G.tensor_tensor(out=m[:], in0=rss[:, 0, :],
                                    in1=rss[:, 1, :], op=sub)
                    G.tensor_tensor(out=t24[:], in0=p[:], in1=m[:], op=mul)
                    G.tensor_tensor(out=o[:, 24, :], in0=t24[:],
                                    in1=c_half[:].to_broadcast([128, c]),
                                    op=mul)
                else:
                    V.scalar_tensor_tensor(o[:, 20, :], rs[:, 0, :],
                                           S15 / 3.0, rs[:, 1, :],
                                           op0=mul, op1=mul)
                    V.scalar_tensor_tensor(o[:, 21, :], rs[:, 1, :],
                                           S15 / 3.0, rs[:, 2, :],
                                           op0=mul, op1=mul)
                    V.scalar_tensor_tensor(o[:, 23, :], rs[:, 0, :],
                                           S15 / 3.0, rs[:, 2, :],
                                           op0=mul, op1=mul)
                    zz = wkp.tile([128, c], f16, tag="zz")
                    p = wkp.tile([128, c], f16, tag="p")
                    m = wkp.tile([128, c], f16, tag="m")
                    V.tensor_tensor(out=zz[:], in0=rs[:, 2, :],
                                    in1=rs[:, 2, :], op=mul)
                    V.tensor_scalar(o[:, 22, :], zz[:], 0.5 * S5, -0.5 * S5,
                                    op0=mul, op1=add)
                    V.tensor_tensor(out=p[:], in0=rs[:, 0, :],
                                    in1=rs[:, 1, :], op=add)
                    V.tensor_tensor(out=m[:], in0=rs[:, 0, :],
                                    in1=rs[:, 1, :], op=sub)
                    V.scalar_tensor_tensor(o[:, 24, :], p[:], 0.5 * S15 / 3.0,
                                           m[:], op0=mul, op1=mul)
                nc.sync.dma_start(
                    out=out_d.ap()[t, :, i0 * 25:(i0 + c) * 25],
                    in_=o[:].rearrange("p k c -> p (k c)"))

            def body():
                prev = None
                for t in range(3):
                    for (i0, c) in chunks:
                        if prev is not None:
                            back(*prev)
                        prev = (t, i0, c, front(t, i0, c))
                back(*prev)

            use_body = dma_body if dma_only else body
            if repeat == 1:
                use_body()
            elif unroll:
                for _ in range(repeat):
                    use_body()
            else:
                with tc.For_i(0, repeat):
                    use_body()
    nc.compile()
    return nc


def _seg_grids(trans_g, f_src, t_dst, W):
    """Host marshaling: place trans[f_src] rows + mask into padded per-core
    channel-planar segment grids [N_CORES, 128, 4, SEG_PER_PART*W]."""
    n = f_src.shape[0]
    order = np.argsort(t_dst, kind="stable")
    sd = t_dst[order]
    sf = f_src[order]
    starts = np.searchsorted(sd, np.arange(N_TFN))
    rank = np.arange(n) - starts[sd]
    core = sd // SEG_PER_CORE
    local = sd % SEG_PER_CORE
    p = local // SEG_PER_PART
    j = local % SEG_PER_PART
    FW = SEG_PER_PART * W
    grids = np.zeros((N_CORES, 128, 4, FW), np.float32)
    vals = trans_g[sf]  # [n, 3]
    pos = j * W + rank
    grids[core, p, 0, pos] = vals[:, 0]
    grids[core, p, 1, pos] = vals[:, 1]
    grids[core, p, 2, pos] = vals[:, 2]
    grids[core, p, 3, pos] = 1.0
    return grids


def _edge_grid_ab(rows_a, rows_b):
    """[E, 3] a-rows and b-rows -> per-core [128, CP*6] grids laid out as
    per-chunk planar [128, 6, c] blocks (matching the device AP)."""
    out = np.empty((N_CORES, 128, CP * 6), np.float32)
    for k in range(N_CORES):
        pad = np.zeros((EPC_PAD, 6), np.float32)
        pad[:EDGES_PER_CORE, 0:3] = rows_a[k * EDGES_PER_CORE:(k + 1) * EDGES_PER_CORE]
        pad[:EDGES_PER_CORE, 3:6] = rows_b[k * EDGES_PER_CORE:(k + 1) * EDGES_PER_CORE]
        g = pad.reshape(128, CP, 6)
        i0 = 0
        for c in CHUNKS:
            blk = g[:, i0:i0 + c, :].transpose(0, 2, 1)  # [128, 6, c]
            out[k][:, i0 * 6:(i0 + c) * 6] = blk.reshape(128, 6 * c)
            i0 += c
    return out


def marshal_a(trans, f_src, t_dst):
    cnts = np.bincount(t_dst, minlength=N_TFN)
    W = int(cnts.max())
    grids = _seg_grids(trans, f_src, t_dst, W)
    in_maps = [{"grid": grids[k].reshape(128, 4, SEG_PER_PART * W)}
               for k in range(N_CORES)]
    return W, in_maps


def unshard_a(results):
    tfn_x = np.zeros((SEG_PAD, 3), np.float32)
    for k in range(N_CORES):
        o = results[k]["tfn"].reshape(128, 3, SEG_PER_PART)
        segs = (np.arange(128)[:, None] * SEG_PER_PART
                + np.arange(SEG_PER_PART)[None, :] + k * SEG_PER_CORE)
        tfn_x[segs.ravel()] = o.transpose(0, 2, 1).reshape(-1, 3)
    return tfn_x[:N_TFN]


def marshal_b(trans, tfn_x, f2t, t2t, t2f):
    f_src, t_dst = f2t[0], f2t[1]
    ab0 = _edge_grid_ab(trans[f_src], tfn_x[t_dst])
    ab1 = _edge_grid_ab(tfn_x[t2t[0]], tfn_x[t2t[1]])
    ab2 = _edge_grid_ab(tfn_x[t2f[0]], trans[t2f[1]])
    mu_grid = np.broadcast_to((MU / SIGMA)[None, :], (128, NUM_RBF)).copy()
    return [{"ab0": ab0[k], "ab1": ab1[k], "ab2": ab2[k], "mu": mu_grid}
            for k in range(N_CORES)]


def unshard_b(results):
    out = np.empty((3, E, NUM_RBF + 9), np.float32)
    for k in range(N_CORES):
        o = results[k]["out"].astype(np.float32)  # [3, 128, CP*25]
        full = np.empty((3, 128, CP, 25), np.float32)
        i0 = 0
        for c in CHUNKS:
            blk = o[:, :, i0 * 25:(i0 + c) * 25].reshape(3, 128, 25, c)
            full[:, :, i0:i0 + c, :] = blk.transpose(0, 1, 3, 2)
            i0 += c
        out[:, k * EDGES_PER_CORE:(k + 1) * EDGES_PER_CORE, :] = \
            full.reshape(3, EPC_PAD, 25)[:, :EDGES_PER_CORE, :]
    return out


def kernel(trans, frame2tfn_edge_index, tfn2tfn_edge_index,
           tfn2frame_edge_index, n_tfn):
    trans = np.asarray(trans, np.float32)
    f2t = np.asarray(frame2tfn_edge_index, np.int64)
    t2t = np.asarray(tfn2tfn_edge_index, np.int64)
    t2f = np.asarray(tfn2frame_edge_index, np.int64)

    # ---- Launch A: scatter-mean ----
    W, in_maps_a = marshal_a(trans, f2t[0], f2t[1])
    key = ("A", W, 1)
    if key not in _cache:
        _cache[key] = _build_launch_a(W)
    ncA = _cache[key]
    resA = bass_utils.run_bass_kernel_spmd(ncA, in_maps_a,
                                           core_ids=list(range(N_CORES)))
    tfn_x = unshard_a(resA.results)

    # ---- Launch B: features ----
    in_maps_b = marshal_b(trans, tfn_x, f2t, t2t, t2f)
    if ("B", 1) not in _cache:
        _cache[("B", 1)] = _build_launch_b()
    ncB = _cache[("B", 1)]
    resB = bass_utils.run_bass_kernel_spmd(ncB, in_maps_b,
                                           core_ids=list(range(N_CORES)))
    return unshard_b(resB.results)
